# revision 9
# baseline (speedup 1.0000x reference)
"""Trainium2 Bass kernel for nn_CICDM_Net (8-core SPMD).

Sharding: hybrid.
  - Ragged phase (gathers + per-student reductions): data-parallel over the
    B=512 students (64 per core).  Per-core indirect-DMA gathers of
    exer_conc_w / exer_conc_adj / exer_pote_w rows for the 128 log entries of
    each student (L=128 lands on the SBUF partition axis), then per-student
    matmuls contract over L producing Wsum/xw/denB/numB directly in
    [concept, student] (transposed) layout.
  - A = (A1 @ expW) / (valid @ expW) via PE matmuls against exp(conc_conc_w)
    (the reference's column-max shift cancels in the ratio).
  - One AllGather of A^T / Bmat^T (139 KB per core).
  - Table phase + prediction heads: tensor-parallel over E=10000 exercises
    (1250 rows per core).  lambd/guess/slide and the final affine
    Y = g + (1-sl-g)*Y_ are folded into W2^T / D2^T, so the heads are pure
    PSUM-accumulated matmuls; the +g term is one K=1 matmul against a ones
    vector.  (The reference's clip(Y_, 1e-8, 1-1e-8) is skipped: Y_ ∈ [0,1]
    mathematically, so it only changes values at rel ~1e-7.)
"""

import sys

import numpy as np

sys.path.insert(0, "/opt/trn_rl_repo")

import concourse.bass as bass  # noqa: E402
import concourse.bacc as bacc  # noqa: E402
import concourse.tile as tile  # noqa: E402
from concourse import bass_utils, mybir  # noqa: E402
from concourse.masks import make_identity  # noqa: E402

F32 = mybir.dt.float32
I32 = mybir.dt.int32
I16 = mybir.dt.int16
AF_ = mybir.ActivationFunctionType
OP = mybir.AluOpType

B, L, C, E, P = 512, 128, 512, 10000, 32
NC_ = 8
BS = B // NC_          # students per core
ES = E // NC_          # exercise rows per core
S = 4                  # students per gather group
NG = BS // S           # gather groups
CCH = C // 128         # concept chunks of 128
NT = (ES + 127) // 128  # table tiles (last one ragged: 98 rows)
EPS = 1e-30
# combined gather table row: [cw(512) | adj(512) | pote(32) | pad(32)]
CW0, ADJ0, POTE0, ROWW = 0, C, 2 * C, 2 * C + P + 32

_nc_cache = [None]


def _build():
    nc = bacc.Bacc("TRN2", target_bir_lowering=False, debug=False,
                   enable_asserts=False, num_devices=NC_)

    # ---- per-core inputs ----
    comb = nc.dram_tensor("comb", [E, ROWW], F32, kind="ExternalInput")
    idxw = nc.dram_tensor("idxw", [128, BS * L // 16], I16,
                          kind="ExternalInput")
    ms_t = nc.dram_tensor("ms_t", [L, 2 * BS], F32, kind="ExternalInput")
    cw_sh = nc.dram_tensor("cw_sh", [ES, C], F32, kind="ExternalInput")
    adj_sh = nc.dram_tensor("adj_sh", [ES, C], F32, kind="ExternalInput")
    pote_sh = nc.dram_tensor("pote_sh", [ES, P], F32, kind="ExternalInput")
    lam_sh = nc.dram_tensor("lam_sh", [1, ES], F32, kind="ExternalInput")
    gue_sh = nc.dram_tensor("gue_sh", [1, ES], F32, kind="ExternalInput")
    sli_sh = nc.dram_tensor("sli_sh", [1, ES], F32, kind="ExternalInput")
    wcc = nc.dram_tensor("wcc", [C, C], F32, kind="ExternalInput")

    # ---- per-core outputs ----
    a_out = nc.dram_tensor("A_out", [BS, C], F32, kind="ExternalOutput")
    y_out = nc.dram_tensor("Y_out", [B, ES], F32, kind="ExternalOutput")

    with tile.TileContext(nc) as tc:
        with tc.tile_pool(name="const", bufs=1) as constp, \
             tc.tile_pool(name="big", bufs=1) as bigp, \
             tc.tile_pool(name="gat", bufs=2) as gatp, \
             tc.tile_pool(name="tab", bufs=2) as tabp, \
             tc.tile_pool(name="small", bufs=3) as smallp, \
             tc.tile_pool(name="psum", bufs=1, space="PSUM") as psp, \
             tc.tile_pool(name="dram", bufs=1, space="DRAM") as dramp:

            # ================= constants / small inputs =================
            ident = constp.tile([128, 128], F32, name="ident")
            make_identity(nc, ident[:])
            ones1 = constp.tile([1, 128], F32, name="ones1")
            nc.vector.memset(ones1[:], 1.0)

            idx_sb = constp.tile([128, BS * L // 16], I16, name="idx_sb")
            nc.sync.dma_start(idx_sb[:], idxw[:])
            ms_sb = constp.tile([L, 2 * BS], F32, name="ms_sb")
            nc.sync.dma_start(ms_sb[:], ms_t[:])
            # s~ = scores * mask  (odd cols *= even cols)
            nc.vector.tensor_tensor(out=ms_sb[:, 1::2], in0=ms_sb[:, 1::2],
                                    in1=ms_sb[:, 0::2], op=OP.mult)

            # per-e vectors in [128, NT] layout (col t = rows t*128.. of shard)
            def load_vec(name, src):
                v = constp.tile([128, NT], F32, name=name)
                nc.vector.memset(v[:], 0.0)
                full_t = ES // 128  # 9 full tiles
                nc.sync.dma_start(
                    v[:, :full_t],
                    src[:, :full_t * 128].rearrange("1 (t p) -> p t", p=128))
                rem = ES - full_t * 128
                if rem:
                    nc.sync.dma_start(
                        v[:rem, full_t:full_t + 1],
                        src[:, full_t * 128:].rearrange("1 p -> p 1"))
                nc.scalar.activation(v[:], v[:], AF_.Sigmoid)
                return v

            lam_v = load_vec("lam_v", lam_sh)
            gue_v = load_vec("gue_v", gue_sh)
            sli_v = load_vec("sli_v", sli_sh)
            # coefA = (1-sl-g)*(1-lam) ; coefB = (1-sl-g)*lam
            t1 = constp.tile([128, NT], F32, name="t1")
            nc.vector.tensor_scalar(t1[:], sli_v[:], -1.0, 1.0,
                                    op0=OP.mult, op1=OP.add)       # 1-sl
            nc.vector.tensor_tensor(out=t1[:], in0=t1[:], in1=gue_v[:],
                                    op=OP.subtract)                # 1-sl-g
            coefA_v = constp.tile([128, NT], F32, name="coefA_v")
            nc.vector.tensor_scalar(coefA_v[:], lam_v[:], -1.0, 1.0,
                                    op0=OP.mult, op1=OP.add)       # 1-lam
            nc.vector.tensor_tensor(out=coefA_v[:], in0=coefA_v[:], in1=t1[:],
                                    op=OP.mult)
            coefB_v = constp.tile([128, NT], F32, name="coefB_v")
            nc.vector.tensor_tensor(out=coefB_v[:], in0=lam_v[:], in1=t1[:],
                                    op=OP.mult)

            g_row = constp.tile([1, ES], F32, name="g_row")
            nc.sync.dma_start(g_row[:], gue_sh[:])
            nc.scalar.activation(g_row[:], g_row[:], AF_.Sigmoid)

            # ================= phase G: ragged gathers ==================
            # psum accumulators held through the phase
            pg_a = psp.tile([128, 4 * 128], F32, name="pg_a")   # [c, ci*128+2b{+1}]
            pg_b = psp.tile([P, 2 * BS], F32, name="pg_b")      # [p, 2b{+1}]

            nidx_g = S * L                      # indices per gather group
            for g in range(NG):
                gt = gatp.tile([128, S * ROWW], F32, tag="gt")
                g3 = gt[:].rearrange("p (s w) -> p s w", w=ROWW)
                nc.gpsimd.dma_gather(
                    out_ap=g3, in_ap=comb[:],
                    idxs_ap=idx_sb[:, g * (nidx_g // 16):
                                   (g + 1) * (nidx_g // 16)],
                    num_idxs=nidx_g, num_idxs_reg=nidx_g, elem_size=ROWW)
                # We = sigmoid(cw) * adj ; expD = exp(pote)   (3D strided views)
                nc.scalar.activation(g3[:, :, CW0:CW0 + C],
                                     g3[:, :, CW0:CW0 + C], AF_.Sigmoid)
                nc.vector.tensor_tensor(out=g3[:, :, CW0:CW0 + C],
                                        in0=g3[:, :, CW0:CW0 + C],
                                        in1=g3[:, :, ADJ0:ADJ0 + C],
                                        op=OP.mult)
                nc.scalar.activation(g3[:, :, POTE0:POTE0 + P],
                                     g3[:, :, POTE0:POTE0 + P], AF_.Exp)

                for s in range(S):
                    b = g * S + s
                    rhs = ms_sb[:, 2 * b:2 * b + 2]
                    for ci in range(CCH):
                        nc.tensor.matmul(
                            pg_a[:, ci * 128 + 2 * b:ci * 128 + 2 * b + 2],
                            lhsT=gt[:, s * ROWW + ci * 128:
                                    s * ROWW + (ci + 1) * 128],
                            rhs=rhs, start=True, stop=True)
                    nc.tensor.matmul(pg_b[:, 2 * b:2 * b + 2],
                                     lhsT=gt[:, s * ROWW + POTE0:
                                             s * ROWW + POTE0 + P],
                                     rhs=rhs, start=True, stop=True)

            # A1^T / valid^T per concept chunk  -> rhsC[ci] = [A1T | validT]
            rhsC = [bigp.tile([128, 128], F32, name=f"rhsC{ci}")
                    for ci in range(CCH)]
            for ci in range(CCH):
                wst = pg_a[:, ci * 128:(ci + 1) * 128:2]
                xwt = pg_a[:, ci * 128 + 1:(ci + 1) * 128:2]
                tmp = smallp.tile([128, BS], F32, tag="tmpA")
                nc.vector.tensor_scalar(tmp[:], wst, EPS, None, op0=OP.max)
                nc.vector.reciprocal(tmp[:], tmp[:])
                nc.vector.tensor_tensor(out=rhsC[ci][:, :BS], in0=xwt,
                                        in1=tmp[:], op=OP.mult)
                nc.vector.tensor_scalar(rhsC[ci][:, BS:], wst, 0.0, None,
                                        op0=OP.is_gt)

            # Bmat^T local [P, BS]
            bm_loc = bigp.tile([P, BS], F32, name="bm_loc")
            tmpb = smallp.tile([P, BS], F32, tag="tmpB")
            nc.vector.tensor_scalar(tmpb[:], pg_b[:, 0::2], EPS, None,
                                    op0=OP.max)
            nc.vector.reciprocal(tmpb[:], tmpb[:])
            nc.vector.tensor_tensor(out=bm_loc[:], in0=pg_b[:, 1::2],
                                    in1=tmpb[:], op=OP.mult)

            # ================= table phase (E-shard) ====================
            w2t = [bigp.tile([128, ES], F32, name=f"w2t{ci}")
                   for ci in range(CCH)]
            d2t = bigp.tile([P, ES], F32, name="d2t")

            for t in range(NT):
                r0 = t * 128
                rn = min(128, ES - r0)
                cw_t = tabp.tile([128, C], F32, tag="cw_t")
                adj_t = tabp.tile([128, C], F32, tag="adj_t")
                if rn < 128:
                    nc.vector.memset(cw_t[:], 0.0)
                    nc.vector.memset(adj_t[:], 0.0)
                nc.sync.dma_start(cw_t[:rn], cw_sh[r0:r0 + rn])
                nc.sync.dma_start(adj_t[:rn], adj_sh[r0:r0 + rn])
                nc.scalar.activation(cw_t[:], cw_t[:], AF_.Sigmoid)
                nc.vector.tensor_tensor(out=cw_t[:], in0=cw_t[:], in1=adj_t[:],
                                        op=OP.mult)               # W rows
                rs = smallp.tile([128, 1], F32, tag="rs")
                nc.vector.tensor_reduce(rs[:], cw_t[:],
                                        axis=mybir.AxisListType.X, op=OP.add)
                nc.vector.tensor_scalar(rs[:], rs[:], EPS, None, op0=OP.max)
                nc.vector.reciprocal(rs[:], rs[:])
                nc.vector.tensor_tensor(out=rs[:], in0=rs[:],
                                        in1=coefA_v[:, t:t + 1], op=OP.mult)
                nc.vector.tensor_scalar(cw_t[:], cw_t[:], rs[:, :1], None,
                                        op0=OP.mult)              # W2'' rows
                for ci in range(CCH):
                    tr = psp.tile([128, 128], F32, tag="tr", bufs=2)
                    nc.tensor.transpose(tr[:], cw_t[:, ci * 128:(ci + 1) * 128],
                                        ident[:])
                    nc.scalar.copy(w2t[ci][:, r0:r0 + rn], tr[:, :rn])

                pote_t = tabp.tile([128, P], F32, tag="pote_t")
                if rn < 128:
                    nc.vector.memset(pote_t[:], 0.0)
                nc.sync.dma_start(pote_t[:rn], pote_sh[r0:r0 + rn])
                rsb = smallp.tile([128, 1], F32, tag="rsb")
                nc.scalar.activation(pote_t[:], pote_t[:], AF_.Exp,
                                     accum_out=rsb[:])
                nc.vector.tensor_scalar(rsb[:], rsb[:], EPS, None, op0=OP.max)
                nc.vector.reciprocal(rsb[:], rsb[:])
                nc.vector.tensor_tensor(out=rsb[:], in0=rsb[:],
                                        in1=coefB_v[:, t:t + 1], op=OP.mult)
                nc.vector.tensor_scalar(pote_t[:], pote_t[:], rsb[:, :1], None,
                                        op0=OP.mult)              # D2'' rows
                trb = psp.tile([P, 128], F32, tag="tr", bufs=2)
                nc.tensor.transpose(trb[:], pote_t[:], ident[:])
                nc.scalar.copy(d2t[:, r0:r0 + rn], trb[:, :rn])

            # ================= phase C: A = num/den =====================
            expw = [bigp.tile([128, C], F32, name=f"expw{ci}")
                    for ci in range(CCH)]
            for ci in range(CCH):
                nc.sync.dma_start(expw[ci][:], wcc[ci * 128:(ci + 1) * 128])
                nc.scalar.activation(expw[ci][:], expw[ci][:], AF_.Exp)

            agin = dramp.tile([(C + P), BS], F32, name="agin")
            a_sb = bigp.tile([BS, C], F32, name="a_sb")
            for di in range(CCH):
                nd = psp.tile([128, 128], F32, tag="nd", bufs=2)
                for ci in range(CCH):
                    nc.tensor.matmul(nd[:],
                                     lhsT=expw[ci][:, di * 128:(di + 1) * 128],
                                     rhs=rhsC[ci][:],
                                     start=(ci == 0), stop=(ci == CCH - 1))
                den = smallp.tile([128, BS], F32, tag="den")
                nc.vector.tensor_scalar(den[:], nd[:, BS:], EPS, None,
                                        op0=OP.max)
                nc.vector.reciprocal(den[:], den[:])
                at_loc = smallp.tile([128, BS], F32, tag="at_loc")
                nc.vector.tensor_tensor(out=at_loc[:], in0=nd[:, :BS],
                                        in1=den[:], op=OP.mult)
                nc.sync.dma_start(agin[di * 128:(di + 1) * 128, :], at_loc[:])
                # transpose for the A output
                tra = psp.tile([BS, 128], F32, tag="tr", bufs=2)
                nc.tensor.transpose(tra[:], at_loc[:], ident[:])
                nc.scalar.copy(a_sb[:, di * 128:(di + 1) * 128], tra[:])
            nc.sync.dma_start(agin[C:, :], bm_loc[:])
            nc.sync.dma_start(a_out[:], a_sb[:])

            # ================= AllGather ================================
            agout = dramp.tile([NC_ * (C + P), BS], F32, name="agout")
            nc.gpsimd.collective_compute(
                "AllGather", OP.bypass,
                replica_groups=[list(range(NC_))],
                ins=[agin[:].opt()], outs=[agout[:].opt()])

            # view: [x, r, b] with x in [0, 544); free order (r, b) matches
            # the desired [c, b_global] SBUF layout.
            agv = agout[:].rearrange("(r x) b -> x r b", r=NC_)
            af = [bigp.tile([128, B], F32, name=f"af{ci}") for ci in range(CCH)]
            for ci in range(CCH):
                nc.sync.dma_start(
                    af[ci][:].rearrange("p (r b) -> p r b", r=NC_),
                    agv[ci * 128:(ci + 1) * 128])
            bf = bigp.tile([P, B], F32, name="bf")
            nc.sync.dma_start(bf[:].rearrange("p (r b) -> p r b", r=NC_),
                              agv[C:])

            # ================= heads ====================================
            e_chunks = []
            e0 = 0
            while e0 < ES:
                en = min(512, ES - e0)
                e_chunks.append((e0, en))
                e0 += en
            for bi in range(B // 128):
                for (e0, en) in e_chunks:
                    hp = psp.tile([128, 512], F32, tag="hp", bufs=2)
                    for ci in range(CCH):
                        nc.tensor.matmul(
                            hp[:, :en],
                            lhsT=af[ci][:, bi * 128:(bi + 1) * 128],
                            rhs=w2t[ci][:, e0:e0 + en],
                            start=(ci == 0), stop=False)
                    nc.tensor.matmul(hp[:, :en],
                                     lhsT=bf[:, bi * 128:(bi + 1) * 128],
                                     rhs=d2t[:, e0:e0 + en],
                                     start=False, stop=False)
                    nc.tensor.matmul(hp[:, :en], lhsT=ones1[:],
                                     rhs=g_row[:, e0:e0 + en],
                                     start=False, stop=True)
                    ysb = tabp.tile([128, 512], F32, tag="ysb")
                    nc.scalar.copy(ysb[:, :en], hp[:, :en])
                    nc.sync.dma_start(y_out[bi * 128:(bi + 1) * 128,
                                            e0:e0 + en], ysb[:, :en])

    nc.compile()
    return nc


def _shard(inputs):
    scores = np.asarray(inputs["scores"], dtype=np.float32)
    adj = np.ascontiguousarray(np.asarray(inputs["exer_conc_adj"],
                                          dtype=np.float32))
    cw = np.ascontiguousarray(np.asarray(inputs["exer_conc_w"],
                                         dtype=np.float32))
    wcc = np.ascontiguousarray(np.asarray(inputs["conc_conc_w"],
                                          dtype=np.float32))
    pote = np.ascontiguousarray(np.asarray(inputs["exer_pote_w"],
                                           dtype=np.float32))
    lambd = np.asarray(inputs["lambd"], dtype=np.float32).reshape(1, E)
    guess = np.asarray(inputs["guess"], dtype=np.float32).reshape(1, E)
    slide = np.asarray(inputs["slide"], dtype=np.float32).reshape(1, E)
    mask = np.asarray(inputs["mask"]).astype(np.float32)
    ids = np.asarray(inputs["exer_ids"]).astype(np.int64)

    # combined gather table: [cw | adj | pote | pad]
    comb = np.zeros((E, ROWW), dtype=np.float32)
    comb[:, CW0:CW0 + C] = cw
    comb[:, ADJ0:ADJ0 + C] = adj
    comb[:, POTE0:POTE0 + P] = pote

    in_maps = []
    for k in range(NC_):
        bsl = slice(k * BS, (k + 1) * BS)
        esl = slice(k * ES, (k + 1) * ES)
        # dma_gather wrapped index layout: flat index j (= student*L + l)
        # lives at [j % 16, j // 16], replicated to all 128 partitions.
        flat = ids[bsl].ravel().astype(np.int16)
        iw = np.ascontiguousarray(
            flat.reshape(-1, 16).T)                     # [16, BS*L/16]
        iw = np.tile(iw, (8, 1))                        # [128, BS*L/16]
        ms_t = np.empty((L, 2 * BS), dtype=np.float32)
        ms_t[:, 0::2] = mask[bsl].T
        ms_t[:, 1::2] = scores[bsl].T
        in_maps.append({
            "comb": comb, "idxw": iw,
            "ms_t": np.ascontiguousarray(ms_t),
            "cw_sh": np.ascontiguousarray(cw[esl]),
            "adj_sh": np.ascontiguousarray(adj[esl]),
            "pote_sh": np.ascontiguousarray(pote[esl]),
            "lam_sh": np.ascontiguousarray(lambd[:, esl]),
            "gue_sh": np.ascontiguousarray(guess[:, esl]),
            "sli_sh": np.ascontiguousarray(slide[:, esl]),
            "wcc": wcc,
        })
    return in_maps


def get_nc():
    if _nc_cache[0] is None:
        _nc_cache[0] = _build()
    return _nc_cache[0]


def run_spmd(inputs, **kw):
    nc = get_nc()
    in_maps = _shard(inputs)
    return bass_utils.run_bass_kernel_spmd(nc, in_maps,
                                           core_ids=list(range(NC_)), **kw)


def assemble(results):
    A = np.concatenate([results[k]["A_out"] for k in range(NC_)], axis=0)
    Y = np.concatenate([results[k]["Y_out"] for k in range(NC_)], axis=1)
    return A, Y


def kernel(**inputs):
    res = run_spmd(inputs)
    return assemble(res.results)


# revision 11
# speedup vs baseline: 1.6490x; 1.6490x over previous
"""Trainium2 Bass kernel for nn_CICDM_Net (8-core SPMD).

Sharding: hybrid.
  - Ragged phase (gathers + per-student reductions): data-parallel over the
    B=512 students (64 per core).  One dma_gather per 8-student group fetches
    the combined [cw | adj | pote] rows for the 128 log entries of each
    student (L=128 on the SBUF partition axis), then per-student matmuls
    contract over L producing Wsum/xw/denB/numB directly in
    [concept, student] (transposed) layout.
  - A = (A1 @ expW) / (valid @ expW) via PE matmuls against exp(conc_conc_w)
    (the reference's column-max shift cancels in the ratio).
  - One AllGather of A^T / Bmat^T across the 8 cores.
  - Table phase + prediction heads: tensor-parallel over E=10000 exercises
    (1250 rows per core).  lambd/guess/slide and the final affine
    Y = g + (1-sl-g)*Y_ are folded into W2^T / D2^T, so the heads are pure
    PSUM-accumulated matmuls; the +g term is one K=1 matmul against a ones
    vector.  (The reference's clip(Y_, 1e-8, 1-1e-8) is skipped: Y_ ∈ [0,1]
    mathematically, so it only changes values at rel ~1e-7.)

Precision: gather table / phase-G matmuls / heads run in bf16 (PSUM
accumulation is f32); phase C and the A output stay f32.
"""

import sys

import numpy as np

sys.path.insert(0, "/opt/trn_rl_repo")

import concourse.bass as bass  # noqa: E402
import concourse.bacc as bacc  # noqa: E402
import concourse.tile as tile  # noqa: E402
from concourse import bass_utils, mybir  # noqa: E402
from concourse.masks import make_identity  # noqa: E402

F32 = mybir.dt.float32
BF16 = mybir.dt.bfloat16
I16 = mybir.dt.int16
AF_ = mybir.ActivationFunctionType
OP = mybir.AluOpType

USE_BF16 = True
GDT = BF16 if USE_BF16 else F32          # gather/head compute dtype
GDT_NP = np.dtype("bfloat16") if USE_BF16 else np.float32

B, L, C, E, P = 512, 128, 512, 10000, 32
NC_ = 8
BS = B // NC_          # students per core
ES = E // NC_          # exercise rows per core
S = 8                  # students per gather group
NG = BS // S           # gather groups
CCH = C // 128         # concept chunks of 128
NT = (ES + 127) // 128  # table tiles (last one ragged: 98 rows)
EPS = 1e-30
# combined gather-table row: [cw(512) | adj(512) | pote(32) | pad]
CW0, ADJ0, POTE0 = 0, C, 2 * C
ROWW = 1152 if USE_BF16 else 1088   # row length in elements (bytes % 256 == 0)

_nc_cache = [None]


def _build():
    nc = bacc.Bacc("TRN2", target_bir_lowering=False, debug=False,
                   enable_asserts=False, num_devices=NC_)

    # ---- per-core inputs ----
    comb = nc.dram_tensor("comb", [E, ROWW], GDT, kind="ExternalInput")
    idxw = nc.dram_tensor("idxw", [128, BS * L // 16], I16,
                          kind="ExternalInput")
    ms_t = nc.dram_tensor("ms_t", [L, 2 * BS], GDT, kind="ExternalInput")
    cw_sh = nc.dram_tensor("cw_sh", [ES, C], F32, kind="ExternalInput")
    adj_sh = nc.dram_tensor("adj_sh", [ES, C], F32, kind="ExternalInput")
    pote_sh = nc.dram_tensor("pote_sh", [ES, P], F32, kind="ExternalInput")
    lam_sh = nc.dram_tensor("lam_sh", [1, ES], F32, kind="ExternalInput")
    gue_sh = nc.dram_tensor("gue_sh", [1, ES], F32, kind="ExternalInput")
    sli_sh = nc.dram_tensor("sli_sh", [1, ES], F32, kind="ExternalInput")
    wcc = nc.dram_tensor("wcc", [C, C], F32, kind="ExternalInput")

    # ---- per-core outputs ----
    a_out = nc.dram_tensor("A_out", [BS, C], F32, kind="ExternalOutput")
    y_out = nc.dram_tensor("Y_out", [B, ES], F32, kind="ExternalOutput")

    with tile.TileContext(nc) as tc:
        with tc.tile_pool(name="const", bufs=1) as constp, \
             tc.tile_pool(name="big", bufs=1) as bigp, \
             tc.tile_pool(name="gat", bufs=2) as gatp, \
             tc.tile_pool(name="tab", bufs=2) as tabp, \
             tc.tile_pool(name="small", bufs=3) as smallp, \
             tc.tile_pool(name="psum", bufs=1, space="PSUM") as psp, \
             tc.tile_pool(name="dram", bufs=1, space="DRAM") as dramp:

            # ================= constants / small inputs =================
            ident = constp.tile([128, 128], F32, name="ident")
            make_identity(nc, ident[:])
            ones1 = constp.tile([1, 128], GDT, name="ones1")
            nc.vector.memset(ones1[:], 1.0)

            idx_sb = constp.tile([128, BS * L // 16], I16, name="idx_sb")
            nc.sync.dma_start(idx_sb[:], idxw[:])
            ms_sb = constp.tile([L, 2 * BS], GDT, name="ms_sb")
            nc.sync.dma_start(ms_sb[:], ms_t[:])
            # s~ = scores * mask  (odd cols *= even cols)
            nc.vector.tensor_tensor(out=ms_sb[:, 1::2], in0=ms_sb[:, 1::2],
                                    in1=ms_sb[:, 0::2], op=OP.mult)

            # ============ phase G: gathers + sigmoid + We ==============
            # psum accumulators held through the phase
            pg_a = psp.tile([128, 4 * 128], F32, name="pg_a")   # [c, ci*128+2b]
            pg_b = psp.tile([P, 2 * BS], F32, name="pg_b")      # [p, 2b]
            potes = bigp.tile([128, BS * P], GDT, name="potes")  # exp(D) later

            nidx_g = S * L                      # indices per gather group
            gts = []
            for g in range(NG):
                gt = gatp.tile([128, S * ROWW], GDT, tag="gt")
                gts.append(gt)
                g3 = gt[:].rearrange("p (s w) -> p s w", w=ROWW)
                nc.gpsimd.dma_gather(
                    out_ap=g3, in_ap=comb[:],
                    idxs_ap=idx_sb[:, g * (nidx_g // 16):
                                   (g + 1) * (nidx_g // 16)],
                    num_idxs=nidx_g, num_idxs_reg=nidx_g, elem_size=ROWW)
                # stash raw pote rows for one batched Exp later
                nc.vector.tensor_copy(
                    potes[:, g * S * P:(g + 1) * S * P].rearrange(
                        "p (s w) -> p s w", w=P),
                    g3[:, :, POTE0:POTE0 + P])
                # We = sigmoid(cw) * adj   (in place, 3D strided views)
                nc.scalar.activation(g3[:, :, CW0:CW0 + C],
                                     g3[:, :, CW0:CW0 + C], AF_.Sigmoid)
                nc.vector.tensor_tensor(out=g3[:, :, CW0:CW0 + C],
                                        in0=g3[:, :, CW0:CW0 + C],
                                        in1=g3[:, :, ADJ0:ADJ0 + C],
                                        op=OP.mult)
                for s in range(S):
                    b = g * S + s
                    rhs = ms_sb[:, 2 * b:2 * b + 2]
                    for ci in range(CCH):
                        nc.tensor.matmul(
                            pg_a[:, ci * 128 + 2 * b:ci * 128 + 2 * b + 2],
                            lhsT=gt[:, s * ROWW + ci * 128:
                                    s * ROWW + (ci + 1) * 128],
                            rhs=rhs, start=True, stop=True)

            # ====== sigmoid-world: per-e vectors, g_row, table W2'' =====
            def load_vec(name, src):
                v = constp.tile([128, NT], F32, name=name)
                nc.vector.memset(v[:], 0.0)
                full_t = ES // 128  # 9 full tiles
                nc.sync.dma_start(
                    v[:, :full_t],
                    src[:, :full_t * 128].rearrange("1 (t p) -> p t", p=128))
                rem = ES - full_t * 128
                if rem:
                    nc.sync.dma_start(
                        v[:rem, full_t:full_t + 1],
                        src[:, full_t * 128:].rearrange("1 p -> p 1"))
                nc.scalar.activation(v[:], v[:], AF_.Sigmoid)
                return v

            lam_v = load_vec("lam_v", lam_sh)
            gue_v = load_vec("gue_v", gue_sh)
            sli_v = load_vec("sli_v", sli_sh)
            # coefA = (1-sl-g)*(1-lam) ; coefB = (1-sl-g)*lam
            t1 = constp.tile([128, NT], F32, name="t1")
            nc.vector.tensor_scalar(t1[:], sli_v[:], -1.0, 1.0,
                                    op0=OP.mult, op1=OP.add)       # 1-sl
            nc.vector.tensor_tensor(out=t1[:], in0=t1[:], in1=gue_v[:],
                                    op=OP.subtract)                # 1-sl-g
            coefA_v = constp.tile([128, NT], F32, name="coefA_v")
            nc.vector.tensor_scalar(coefA_v[:], lam_v[:], -1.0, 1.0,
                                    op0=OP.mult, op1=OP.add)       # 1-lam
            nc.vector.tensor_tensor(out=coefA_v[:], in0=coefA_v[:], in1=t1[:],
                                    op=OP.mult)
            coefB_v = constp.tile([128, NT], F32, name="coefB_v")
            nc.vector.tensor_tensor(out=coefB_v[:], in0=lam_v[:], in1=t1[:],
                                    op=OP.mult)

            g_row32 = constp.tile([1, ES], F32, name="g_row32")
            nc.sync.dma_start(g_row32[:], gue_sh[:])
            nc.scalar.activation(g_row32[:], g_row32[:], AF_.Sigmoid)
            g_row = constp.tile([1, ES], GDT, name="g_row")
            nc.vector.tensor_copy(g_row[:], g_row32[:])

            # table W2'': sigmoid + adj-mask + row-normalize + fold coefA
            w2t = [bigp.tile([128, ES], GDT, name=f"w2t{ci}")
                   for ci in range(CCH)]
            for t in range(NT):
                r0 = t * 128
                rn = min(128, ES - r0)
                cw_t = tabp.tile([128, C], F32, tag="cw_t")
                adj_t = tabp.tile([128, C], F32, tag="adj_t")
                if rn < 128:
                    nc.vector.memset(cw_t[:], 0.0)
                    nc.vector.memset(adj_t[:], 0.0)
                nc.sync.dma_start(cw_t[:rn], cw_sh[r0:r0 + rn])
                nc.sync.dma_start(adj_t[:rn], adj_sh[r0:r0 + rn])
                nc.scalar.activation(cw_t[:], cw_t[:], AF_.Sigmoid)
                nc.vector.tensor_tensor(out=cw_t[:], in0=cw_t[:], in1=adj_t[:],
                                        op=OP.mult)               # W rows
                rs = smallp.tile([128, 1], F32, tag="rs")
                nc.vector.tensor_reduce(rs[:], cw_t[:],
                                        axis=mybir.AxisListType.X, op=OP.add)
                nc.vector.tensor_scalar(rs[:], rs[:], EPS, None, op0=OP.max)
                nc.vector.reciprocal(rs[:], rs[:])
                nc.vector.tensor_tensor(out=rs[:], in0=rs[:],
                                        in1=coefA_v[:, t:t + 1], op=OP.mult)
                nc.vector.tensor_scalar(cw_t[:], cw_t[:], rs[:, :1], None,
                                        op0=OP.mult)              # W2'' rows
                for ci in range(CCH):
                    tr = psp.tile([128, 128], F32, tag="tr", bufs=2)
                    nc.tensor.transpose(tr[:], cw_t[:, ci * 128:(ci + 1) * 128],
                                        ident[:])
                    nc.scalar.copy(w2t[ci][:, r0:r0 + rn], tr[:, :rn])

            # ================= exp-world ================================
            # gathered pote rows -> exp, then the pg_b matmuls
            nc.scalar.activation(potes[:], potes[:], AF_.Exp)
            for b in range(BS):
                nc.tensor.matmul(pg_b[:, 2 * b:2 * b + 2],
                                 lhsT=potes[:, b * P:(b + 1) * P],
                                 rhs=ms_sb[:, 2 * b:2 * b + 2],
                                 start=True, stop=True)

            # expW = exp(conc_conc_w)  [c, d] layout
            expw = [bigp.tile([128, C], F32, name=f"expw{ci}")
                    for ci in range(CCH)]
            for ci in range(CCH):
                nc.sync.dma_start(expw[ci][:], wcc[ci * 128:(ci + 1) * 128])
                nc.scalar.activation(expw[ci][:], expw[ci][:], AF_.Exp)

            # table D2'': exp + row-normalize + fold coefB
            d2t = bigp.tile([P, ES], GDT, name="d2t")
            for t in range(NT):
                r0 = t * 128
                rn = min(128, ES - r0)
                pote_t = tabp.tile([128, P], F32, tag="pote_t")
                if rn < 128:
                    nc.vector.memset(pote_t[:], 0.0)
                nc.sync.dma_start(pote_t[:rn], pote_sh[r0:r0 + rn])
                rsb = smallp.tile([128, 1], F32, tag="rsb")
                nc.scalar.activation(pote_t[:], pote_t[:], AF_.Exp,
                                     accum_out=rsb[:])
                nc.vector.tensor_scalar(rsb[:], rsb[:], EPS, None, op0=OP.max)
                nc.vector.reciprocal(rsb[:], rsb[:])
                nc.vector.tensor_tensor(out=rsb[:], in0=rsb[:],
                                        in1=coefB_v[:, t:t + 1], op=OP.mult)
                nc.vector.tensor_scalar(pote_t[:], pote_t[:], rsb[:, :1], None,
                                        op0=OP.mult)              # D2'' rows
                trb = psp.tile([P, 128], F32, tag="tr", bufs=2)
                nc.tensor.transpose(trb[:], pote_t[:], ident[:])
                nc.scalar.copy(d2t[:, r0:r0 + rn], trb[:, :rn])

            # ============ A1/valid, Bmat, phase C, A output =============
            rhsC = [bigp.tile([128, 128], F32, name=f"rhsC{ci}")
                    for ci in range(CCH)]
            for ci in range(CCH):
                wst = pg_a[:, ci * 128:(ci + 1) * 128:2]
                xwt = pg_a[:, ci * 128 + 1:(ci + 1) * 128:2]
                tmp = smallp.tile([128, BS], F32, tag="tmpA")
                nc.vector.tensor_scalar(tmp[:], wst, EPS, None, op0=OP.max)
                nc.vector.reciprocal(tmp[:], tmp[:])
                nc.vector.tensor_tensor(out=rhsC[ci][:, :BS], in0=xwt,
                                        in1=tmp[:], op=OP.mult)
                nc.vector.tensor_scalar(rhsC[ci][:, BS:], wst, 0.0, None,
                                        op0=OP.is_gt)

            # Bmat^T local [P, BS] (bf16 for the AG payload)
            bm16 = bigp.tile([P, BS], GDT, name="bm16")
            tmpb = smallp.tile([P, BS], F32, tag="tmpB")
            nc.vector.tensor_scalar(tmpb[:], pg_b[:, 0::2], EPS, None,
                                    op0=OP.max)
            nc.vector.reciprocal(tmpb[:], tmpb[:])
            nc.vector.tensor_tensor(out=bm16[:], in0=pg_b[:, 1::2],
                                    in1=tmpb[:], op=OP.mult)

            agin = dramp.tile([(C + P), BS], GDT, name="agin")
            a_sb = bigp.tile([BS, C], F32, name="a_sb")
            for di in range(CCH):
                nd = psp.tile([128, 128], F32, tag="nd", bufs=2)
                for ci in range(CCH):
                    nc.tensor.matmul(nd[:],
                                     lhsT=expw[ci][:, di * 128:(di + 1) * 128],
                                     rhs=rhsC[ci][:],
                                     start=(ci == 0), stop=(ci == CCH - 1))
                den = smallp.tile([128, BS], F32, tag="den")
                nc.vector.tensor_scalar(den[:], nd[:, BS:], EPS, None,
                                        op0=OP.max)
                nc.vector.reciprocal(den[:], den[:])
                at_loc = smallp.tile([128, BS], F32, tag="at_loc")
                nc.vector.tensor_tensor(out=at_loc[:], in0=nd[:, :BS],
                                        in1=den[:], op=OP.mult)
                at16 = smallp.tile([128, BS], GDT, tag="at16")
                nc.vector.tensor_copy(at16[:], at_loc[:])
                nc.sync.dma_start(agin[di * 128:(di + 1) * 128, :], at16[:])
                # transpose for the A output
                tra = psp.tile([BS, 128], F32, tag="tr", bufs=2)
                nc.tensor.transpose(tra[:], at_loc[:], ident[:])
                nc.scalar.copy(a_sb[:, di * 128:(di + 1) * 128], tra[:])
            nc.sync.dma_start(agin[C:, :], bm16[:])
            nc.sync.dma_start(a_out[:], a_sb[:])

            # ================= AllGather ================================
            agout = dramp.tile([NC_ * (C + P), BS], GDT, name="agout")
            nc.gpsimd.collective_compute(
                "AllGather", OP.bypass,
                replica_groups=[list(range(NC_))],
                ins=[agin[:].opt()], outs=[agout[:].opt()])

            # view: [x, r, b] with x in [0, 544); free order (r, b) matches
            # the desired [c, b_global] SBUF layout.
            agv = agout[:].rearrange("(r x) b -> x r b", r=NC_)
            af = [bigp.tile([128, B], GDT, name=f"af{ci}") for ci in range(CCH)]
            for ci in range(CCH):
                nc.sync.dma_start(
                    af[ci][:].rearrange("p (r b) -> p r b", r=NC_),
                    agv[ci * 128:(ci + 1) * 128])
            bf = bigp.tile([P, B], GDT, name="bf")
            nc.sync.dma_start(bf[:].rearrange("p (r b) -> p r b", r=NC_),
                              agv[C:])

            # ================= heads ====================================
            e_chunks = []
            e0 = 0
            while e0 < ES:
                en = min(512, ES - e0)
                e_chunks.append((e0, en))
                e0 += en
            for bi in range(B // 128):
                for (e0, en) in e_chunks:
                    hp = psp.tile([128, 512], F32, tag="hp", bufs=2)
                    for ci in range(CCH):
                        nc.tensor.matmul(
                            hp[:, :en],
                            lhsT=af[ci][:, bi * 128:(bi + 1) * 128],
                            rhs=w2t[ci][:, e0:e0 + en],
                            start=(ci == 0), stop=False)
                    nc.tensor.matmul(hp[:, :en],
                                     lhsT=bf[:, bi * 128:(bi + 1) * 128],
                                     rhs=d2t[:, e0:e0 + en],
                                     start=False, stop=False)
                    nc.tensor.matmul(hp[:, :en], lhsT=ones1[:],
                                     rhs=g_row[:, e0:e0 + en],
                                     start=False, stop=True)
                    ysb = tabp.tile([128, 512], F32, tag="ysb")
                    nc.scalar.copy(ysb[:, :en], hp[:, :en])
                    nc.sync.dma_start(y_out[bi * 128:(bi + 1) * 128,
                                            e0:e0 + en], ysb[:, :en])

    nc.compile()
    return nc


def _shard(inputs):
    scores = np.asarray(inputs["scores"], dtype=np.float32)
    adj = np.ascontiguousarray(np.asarray(inputs["exer_conc_adj"],
                                          dtype=np.float32))
    cw = np.ascontiguousarray(np.asarray(inputs["exer_conc_w"],
                                         dtype=np.float32))
    wcc = np.ascontiguousarray(np.asarray(inputs["conc_conc_w"],
                                          dtype=np.float32))
    pote = np.ascontiguousarray(np.asarray(inputs["exer_pote_w"],
                                           dtype=np.float32))
    lambd = np.asarray(inputs["lambd"], dtype=np.float32).reshape(1, E)
    guess = np.asarray(inputs["guess"], dtype=np.float32).reshape(1, E)
    slide = np.asarray(inputs["slide"], dtype=np.float32).reshape(1, E)
    mask = np.asarray(inputs["mask"]).astype(np.float32)
    ids = np.asarray(inputs["exer_ids"]).astype(np.int64)

    # combined gather table: [cw | adj | pote | pad]
    comb = np.zeros((E, ROWW), dtype=GDT_NP)
    comb[:, CW0:CW0 + C] = cw.astype(GDT_NP)
    comb[:, ADJ0:ADJ0 + C] = adj.astype(GDT_NP)
    comb[:, POTE0:POTE0 + P] = pote.astype(GDT_NP)

    in_maps = []
    for k in range(NC_):
        bsl = slice(k * BS, (k + 1) * BS)
        esl = slice(k * ES, (k + 1) * ES)
        # dma_gather wrapped index layout: flat index j (= student*L + l)
        # lives at [j % 16, j // 16], replicated to all 128 partitions.
        flat = ids[bsl].ravel().astype(np.int16)
        iw = np.ascontiguousarray(flat.reshape(-1, 16).T)   # [16, BS*L/16]
        iw = np.tile(iw, (8, 1))                            # [128, BS*L/16]
        ms_t = np.empty((L, 2 * BS), dtype=np.float32)
        ms_t[:, 0::2] = mask[bsl].T
        ms_t[:, 1::2] = scores[bsl].T
        in_maps.append({
            "comb": comb, "idxw": iw,
            "ms_t": np.ascontiguousarray(ms_t).astype(GDT_NP),
            "cw_sh": np.ascontiguousarray(cw[esl]),
            "adj_sh": np.ascontiguousarray(adj[esl]),
            "pote_sh": np.ascontiguousarray(pote[esl]),
            "lam_sh": np.ascontiguousarray(lambd[:, esl]),
            "gue_sh": np.ascontiguousarray(guess[:, esl]),
            "sli_sh": np.ascontiguousarray(slide[:, esl]),
            "wcc": wcc,
        })
    return in_maps


def get_nc():
    if _nc_cache[0] is None:
        _nc_cache[0] = _build()
    return _nc_cache[0]


def run_spmd(inputs, **kw):
    nc = get_nc()
    in_maps = _shard(inputs)
    return bass_utils.run_bass_kernel_spmd(nc, in_maps,
                                           core_ids=list(range(NC_)), **kw)


def assemble(results):
    A = np.concatenate([results[k]["A_out"] for k in range(NC_)], axis=0)
    Y = np.concatenate([results[k]["Y_out"] for k in range(NC_)], axis=1)
    return A, Y


def kernel(**inputs):
    res = run_spmd(inputs)
    return assemble(res.results)


# revision 19
# speedup vs baseline: 1.6989x; 1.0303x over previous
"""Trainium2 Bass kernel for nn_CICDM_Net (8-core SPMD).

Sharding: hybrid.
  - Ragged phase (gathers + per-student reductions): data-parallel over the
    B=512 students (64 per core).  One dma_gather per 8-student group fetches
    the combined [cw | adj | pote] rows for the 128 log entries of each
    student (L=128 on the SBUF partition axis), then per-student matmuls
    contract over L producing Wsum/xw/denB/numB directly in
    [concept, student] (transposed) layout.
  - A = (A1 @ expW) / (valid @ expW) via PE matmuls against exp(conc_conc_w)
    (the reference's column-max shift cancels in the ratio).
  - One AllGather of A^T / Bmat^T across the 8 cores.
  - Table phase + prediction heads: tensor-parallel over E=10000 exercises
    (1250 rows per core).  lambd/guess/slide and the final affine
    Y = g + (1-sl-g)*Y_ are folded into W2^T / D2^T, so the heads are pure
    PSUM-accumulated matmuls; the +g term is one K=1 matmul against a ones
    vector.  (The reference's clip(Y_, 1e-8, 1-1e-8) is skipped: Y_ ∈ [0,1]
    mathematically, so it only changes values at rel ~1e-7.)

Precision: gather table / phase-G matmuls / heads run in bf16 (PSUM
accumulation is f32); phase C and the A output stay f32.
"""

import sys

import numpy as np

sys.path.insert(0, "/opt/trn_rl_repo")

import concourse.bass as bass  # noqa: E402
import concourse.bacc as bacc  # noqa: E402
import concourse.tile as tile  # noqa: E402
from concourse import bass_utils, mybir  # noqa: E402
from concourse.masks import make_identity  # noqa: E402

F32 = mybir.dt.float32
BF16 = mybir.dt.bfloat16
I16 = mybir.dt.int16
AF_ = mybir.ActivationFunctionType
OP = mybir.AluOpType

USE_BF16 = True
GDT = BF16 if USE_BF16 else F32          # gather/head compute dtype
GDT_NP = np.dtype("bfloat16") if USE_BF16 else np.float32

B, L, C, E, P = 512, 128, 512, 10000, 32
NC_ = 8
BS = B // NC_          # students per core
ES = E // NC_          # exercise rows per core
S = 8                  # students per gather group
NG = BS // S           # gather groups
CCH = C // 128         # concept chunks of 128
NT = (ES + 127) // 128  # table tiles (last one ragged: 98 rows)
EPS = 1e-30
# combined gather-table row: [cw(512) | adj(512) | pote(32) | pad]
CW0, ADJ0, POTE0 = 0, C, 2 * C
ROWW = 1152 if USE_BF16 else 1088   # row length in elements (bytes % 256 == 0)

_nc_cache = [None]


def _build():
    nc = bacc.Bacc("TRN2", target_bir_lowering=False, debug=False,
                   enable_asserts=False, num_devices=NC_)

    # ---- per-core inputs ----
    comb = nc.dram_tensor("comb", [E, ROWW], GDT, kind="ExternalInput")
    idxw = nc.dram_tensor("idxw", [128, BS * L // 16], I16,
                          kind="ExternalInput")
    ms_t = nc.dram_tensor("ms_t", [L, 2 * BS], GDT, kind="ExternalInput")
    cw_sh = nc.dram_tensor("cw_sh", [ES, C], F32, kind="ExternalInput")
    adj_sh = nc.dram_tensor("adj_sh", [ES, C], F32, kind="ExternalInput")
    pote_sh = nc.dram_tensor("pote_sh", [ES, P], F32, kind="ExternalInput")
    lam_sh = nc.dram_tensor("lam_sh", [1, ES], F32, kind="ExternalInput")
    gue_sh = nc.dram_tensor("gue_sh", [1, ES], F32, kind="ExternalInput")
    sli_sh = nc.dram_tensor("sli_sh", [1, ES], F32, kind="ExternalInput")
    wcc = nc.dram_tensor("wcc", [C, C], F32, kind="ExternalInput")

    # ---- per-core outputs ----
    # A is written transposed ([C, B]); the host transposes it back.
    a_out = nc.dram_tensor("A_T_out", [C, B], F32, kind="ExternalOutput")
    y_out = nc.dram_tensor("Y_out", [B, ES], F32, kind="ExternalOutput")

    with tile.TileContext(nc) as tc:
        with tc.tile_pool(name="const", bufs=1) as constp, \
             tc.tile_pool(name="big", bufs=1) as bigp, \
             tc.tile_pool(name="gat", bufs=3) as gatp, \
             tc.tile_pool(name="tab", bufs=2) as tabp, \
             tc.tile_pool(name="small", bufs=3) as smallp, \
             tc.tile_pool(name="psum", bufs=1, space="PSUM") as psp, \
             tc.tile_pool(name="dram", bufs=1, space="DRAM") as dramp:

            # ================= constants / small inputs =================
            ident = constp.tile([128, 128], F32, name="ident")
            make_identity(nc, ident[:])
            ones1 = constp.tile([1, 128], GDT, name="ones1")
            nc.vector.memset(ones1[:], 1.0)

            idx_sb = constp.tile([128, BS * L // 16], I16, name="idx_sb")
            nc.sync.dma_start(idx_sb[:], idxw[:])
            ms_sb = constp.tile([L, 2 * BS], GDT, name="ms_sb")
            nc.sync.dma_start(ms_sb[:], ms_t[:])
            # s~ = scores * mask  (odd cols *= even cols)
            nc.vector.tensor_tensor(out=ms_sb[:, 1::2], in0=ms_sb[:, 1::2],
                                    in1=ms_sb[:, 0::2], op=OP.mult)

            # ============ phase G: gathers + sigmoid + We ==============
            # psum accumulators held through the phase
            pg_a = psp.tile([128, 4 * 128], F32, name="pg_a")   # [c, ci*128+2b]
            pg_b = psp.tile([P, 2 * BS], F32, name="pg_b")      # [p, 2b]
            potes = bigp.tile([128, BS * P], GDT, name="potes")  # exp(D) later

            nidx_g = S * L                      # indices per gather group
            gts = []
            for g in range(NG):
                gt = gatp.tile([128, S * ROWW], GDT, tag="gt")
                gts.append(gt)
                g3 = gt[:].rearrange("p (s w) -> p s w", w=ROWW)
                nc.gpsimd.dma_gather(
                    out_ap=g3, in_ap=comb[:],
                    idxs_ap=idx_sb[:, g * (nidx_g // 16):
                                   (g + 1) * (nidx_g // 16)],
                    num_idxs=nidx_g, num_idxs_reg=nidx_g, elem_size=ROWW)
                # stash raw pote rows for one batched Exp later
                nc.vector.tensor_copy(
                    potes[:, g * S * P:(g + 1) * S * P].rearrange(
                        "p (s w) -> p s w", w=P),
                    g3[:, :, POTE0:POTE0 + P])
                # We = sigmoid(cw) * adj   (in place, 3D strided views)
                nc.scalar.activation(g3[:, :, CW0:CW0 + C],
                                     g3[:, :, CW0:CW0 + C], AF_.Sigmoid)
                nc.vector.tensor_tensor(out=g3[:, :, CW0:CW0 + C],
                                        in0=g3[:, :, CW0:CW0 + C],
                                        in1=g3[:, :, ADJ0:ADJ0 + C],
                                        op=OP.mult)
                for s in range(S):
                    b = g * S + s
                    rhs = ms_sb[:, 2 * b:2 * b + 2]
                    for ci in range(CCH):
                        nc.tensor.matmul(
                            pg_a[:, ci * 128 + 2 * b:ci * 128 + 2 * b + 2],
                            lhsT=gt[:, s * ROWW + ci * 128:
                                    s * ROWW + (ci + 1) * 128],
                            rhs=rhs, start=True, stop=True)
                # exp(pote) + pg_b matmuls in two half-batches so they
                # overlap the tail of the gather stream
                if g in (NG // 2 - 1, NG - 1):
                    h0 = 0 if g == NG // 2 - 1 else BS // 2
                    sl = slice(h0 * P, (h0 + BS // 2) * P)
                    nc.scalar.activation(potes[:, sl], potes[:, sl], AF_.Exp)
                    for b in range(h0, h0 + BS // 2):
                        nc.tensor.matmul(pg_b[:, 2 * b:2 * b + 2],
                                         lhsT=potes[:, b * P:(b + 1) * P],
                                         rhs=ms_sb[:, 2 * b:2 * b + 2],
                                         start=True, stop=True)

            # ====== A1/valid + Bmat^T -> early AllGather =================
            rhsC16 = [bigp.tile([128, 128], GDT, name=f"rhsC16_{ci}")
                      for ci in range(CCH)]
            for ci in range(CCH):
                wst = pg_a[:, ci * 128:(ci + 1) * 128:2]
                xwt = pg_a[:, ci * 128 + 1:(ci + 1) * 128:2]
                tmp = smallp.tile([128, BS], F32, tag="tmpA")
                nc.vector.tensor_scalar(tmp[:], wst, EPS, None, op0=OP.max)
                nc.vector.reciprocal(tmp[:], tmp[:])
                nc.vector.tensor_tensor(out=rhsC16[ci][:, :BS], in0=xwt,
                                        in1=tmp[:], op=OP.mult)
                nc.vector.tensor_scalar(rhsC16[ci][:, BS:], wst, 0.0, None,
                                        op0=OP.is_gt)
            bm16 = bigp.tile([P, 128], GDT, name="bm16")
            nc.vector.memset(bm16[:], 0.0)
            tmpb = smallp.tile([P, BS], F32, tag="tmpB")
            nc.vector.tensor_scalar(tmpb[:], pg_b[:, 0::2], EPS, None,
                                    op0=OP.max)
            nc.vector.reciprocal(tmpb[:], tmpb[:])
            nc.vector.tensor_tensor(out=bm16[:, :BS], in0=pg_b[:, 1::2],
                                    in1=tmpb[:], op=OP.mult)

            # AllGather of [A1T|validT] (4x128 rows) + BmatT (32 rows).
            # All DMAs here go through gpsimd (SWDGE) so the blocked queue
            # head doesn't stall the sync-engine HWDGE FIFO that streams
            # the (independent) table loads.
            agin = dramp.tile([(C + P), 128], GDT, name="agin")
            for ci in range(CCH):
                nc.gpsimd.dma_start(agin[ci * 128:(ci + 1) * 128, :],
                                    rhsC16[ci][:])
            nc.gpsimd.dma_start(agin[C:, :], bm16[:])
            agout = dramp.tile([NC_ * (C + P), 128], GDT, name="agout")
            nc.gpsimd.collective_compute(
                "AllGather", OP.bypass,
                replica_groups=[list(range(NC_))],
                ins=[agin[:].opt()], outs=[agout[:].opt()])
            agv = agout[:].rearrange("(r x) j -> x r j", r=NC_)
            a1f = [bigp.tile([128, B], GDT, name=f"a1f{ci}")
                   for ci in range(CCH)]
            vf = [bigp.tile([128, B], GDT, name=f"vf{ci}")
                  for ci in range(CCH)]
            for ci in range(CCH):
                nc.gpsimd.dma_start(
                    a1f[ci][:].rearrange("p (r b) -> p r b", r=NC_),
                    agv[ci * 128:(ci + 1) * 128, :, 0:BS])
                nc.gpsimd.dma_start(
                    vf[ci][:].rearrange("p (r b) -> p r b", r=NC_),
                    agv[ci * 128:(ci + 1) * 128, :, BS:2 * BS])
            bff = bigp.tile([P, B], GDT, name="bff")
            nc.gpsimd.dma_start(bff[:].rearrange("p (r b) -> p r b", r=NC_),
                                agv[C:, :, 0:BS])

            # ====== sigmoid-world: per-e vectors, g_row, table W2'' =====
            def load_vec(name, src):
                v = constp.tile([128, NT], F32, name=name)
                nc.vector.memset(v[:], 0.0)
                full_t = ES // 128  # 9 full tiles
                nc.sync.dma_start(
                    v[:, :full_t],
                    src[:, :full_t * 128].rearrange("1 (t p) -> p t", p=128))
                rem = ES - full_t * 128
                if rem:
                    nc.sync.dma_start(
                        v[:rem, full_t:full_t + 1],
                        src[:, full_t * 128:].rearrange("1 p -> p 1"))
                nc.scalar.activation(v[:], v[:], AF_.Sigmoid)
                return v

            lam_v = load_vec("lam_v", lam_sh)
            gue_v = load_vec("gue_v", gue_sh)
            sli_v = load_vec("sli_v", sli_sh)
            # coefA = (1-sl-g)*(1-lam) ; coefB = (1-sl-g)*lam
            t1 = constp.tile([128, NT], F32, name="t1")
            nc.vector.tensor_scalar(t1[:], sli_v[:], -1.0, 1.0,
                                    op0=OP.mult, op1=OP.add)       # 1-sl
            nc.vector.tensor_tensor(out=t1[:], in0=t1[:], in1=gue_v[:],
                                    op=OP.subtract)                # 1-sl-g
            coefA_v = constp.tile([128, NT], F32, name="coefA_v")
            nc.vector.tensor_scalar(coefA_v[:], lam_v[:], -1.0, 1.0,
                                    op0=OP.mult, op1=OP.add)       # 1-lam
            nc.vector.tensor_tensor(out=coefA_v[:], in0=coefA_v[:], in1=t1[:],
                                    op=OP.mult)
            coefB_v = constp.tile([128, NT], F32, name="coefB_v")
            nc.vector.tensor_tensor(out=coefB_v[:], in0=lam_v[:], in1=t1[:],
                                    op=OP.mult)

            g_row32 = constp.tile([1, ES], F32, name="g_row32")
            nc.sync.dma_start(g_row32[:], gue_sh[:])
            nc.scalar.activation(g_row32[:], g_row32[:], AF_.Sigmoid)
            g_row = constp.tile([1, ES], GDT, name="g_row")
            nc.vector.tensor_copy(g_row[:], g_row32[:])

            # table W2'': sigmoid + adj-mask + row-normalize + fold coefA
            w2t = [bigp.tile([128, ES], GDT, name=f"w2t{ci}")
                   for ci in range(CCH)]
            for t in range(NT):
                r0 = t * 128
                rn = min(128, ES - r0)
                cw_t = tabp.tile([128, C], F32, tag="cw_t")
                adj_t = tabp.tile([128, C], F32, tag="adj_t")
                if rn < 128:
                    nc.vector.memset(cw_t[:], 0.0)
                    nc.vector.memset(adj_t[:], 0.0)
                nc.sync.dma_start(cw_t[:rn], cw_sh[r0:r0 + rn])
                nc.sync.dma_start(adj_t[:rn], adj_sh[r0:r0 + rn])
                nc.scalar.activation(cw_t[:], cw_t[:], AF_.Sigmoid)
                nc.vector.tensor_tensor(out=cw_t[:], in0=cw_t[:], in1=adj_t[:],
                                        op=OP.mult)               # W rows
                rs = smallp.tile([128, 1], F32, tag="rs")
                nc.vector.tensor_reduce(rs[:], cw_t[:],
                                        axis=mybir.AxisListType.X, op=OP.add)
                nc.vector.tensor_scalar(rs[:], rs[:], EPS, None, op0=OP.max)
                nc.vector.reciprocal(rs[:], rs[:])
                nc.vector.tensor_tensor(out=rs[:], in0=rs[:],
                                        in1=coefA_v[:, t:t + 1], op=OP.mult)
                nc.vector.tensor_scalar(cw_t[:], cw_t[:], rs[:, :1], None,
                                        op0=OP.mult)              # W2'' rows
                for ci in range(CCH):
                    tr = psp.tile([128, 128], F32, tag="tr", bufs=2)
                    nc.tensor.transpose(tr[:], cw_t[:, ci * 128:(ci + 1) * 128],
                                        ident[:])
                    nc.scalar.copy(w2t[ci][:, r0:r0 + rn], tr[:, :rn])

            # ================= exp-world ================================
            # expW = exp(conc_conc_w)  [c, d] layout, bf16 for the C matmuls
            expw32 = [tabp.tile([128, C], F32, tag="expw32",
                                name=f"expw32_{ci}")
                      for ci in range(CCH)]
            expw16 = [bigp.tile([128, C], GDT, name=f"expw16_{ci}")
                      for ci in range(CCH)]
            for ci in range(CCH):
                nc.sync.dma_start(expw32[ci][:], wcc[ci * 128:(ci + 1) * 128])
                nc.scalar.activation(expw16[ci][:], expw32[ci][:], AF_.Exp)

            # table D2'': exp + row-normalize + fold coefB
            d2t = bigp.tile([P, ES], GDT, name="d2t")
            for t in range(NT):
                r0 = t * 128
                rn = min(128, ES - r0)
                pote_t = tabp.tile([128, P], F32, tag="pote_t")
                if rn < 128:
                    nc.vector.memset(pote_t[:], 0.0)
                nc.sync.dma_start(pote_t[:rn], pote_sh[r0:r0 + rn])
                rsb = smallp.tile([128, 1], F32, tag="rsb")
                nc.scalar.activation(pote_t[:], pote_t[:], AF_.Exp,
                                     accum_out=rsb[:])
                nc.vector.tensor_scalar(rsb[:], rsb[:], EPS, None, op0=OP.max)
                nc.vector.reciprocal(rsb[:], rsb[:])
                nc.vector.tensor_tensor(out=rsb[:], in0=rsb[:],
                                        in1=coefB_v[:, t:t + 1], op=OP.mult)
                nc.vector.tensor_scalar(pote_t[:], pote_t[:], rsb[:, :1], None,
                                        op0=OP.mult)              # D2'' rows
                trb = psp.tile([P, 128], F32, tag="tr", bufs=2)
                nc.tensor.transpose(trb[:], pote_t[:], ident[:])
                nc.scalar.copy(d2t[:, r0:r0 + rn], trb[:, :rn])

            # ======= phase C, replicated on every core (all 512 b) ======
            af16 = [bigp.tile([128, B], GDT, name=f"af16_{di}")
                    for di in range(CCH)]
            for di in range(CCH):
                ndn = psp.tile([128, B], F32, tag="ndf", bufs=2)
                ndd = psp.tile([128, B], F32, tag="ndf", bufs=2)
                for ci in range(CCH):
                    nc.tensor.matmul(
                        ndn[:], lhsT=expw16[ci][:, di * 128:(di + 1) * 128],
                        rhs=a1f[ci][:], start=(ci == 0), stop=(ci == CCH - 1))
                for ci in range(CCH):
                    nc.tensor.matmul(
                        ndd[:], lhsT=expw16[ci][:, di * 128:(di + 1) * 128],
                        rhs=vf[ci][:], start=(ci == 0), stop=(ci == CCH - 1))
                smc = smallp.tile([128, B], F32, tag="smc")
                nc.vector.tensor_scalar(smc[:], ndd[:], EPS, None, op0=OP.max)
                nc.vector.reciprocal(smc[:], smc[:])
                atf = smallp.tile([128, B], F32, tag="atf")
                nc.vector.tensor_tensor(out=atf[:], in0=ndn[:], in1=smc[:],
                                        op=OP.mult)
                nc.scalar.copy(af16[di][:], atf[:])
                nc.sync.dma_start(a_out[di * 128:(di + 1) * 128, :], atf[:])

            # ================= heads ====================================
            e_chunks = []
            e0 = 0
            while e0 < ES:
                en = min(512, ES - e0)
                e_chunks.append((e0, en))
                e0 += en
            for bi in range(B // 128):
                for (e0, en) in e_chunks:
                    hp = psp.tile([128, 512], F32, tag="hp", bufs=2)
                    for ci in range(CCH):
                        nc.tensor.matmul(
                            hp[:, :en],
                            lhsT=af16[ci][:, bi * 128:(bi + 1) * 128],
                            rhs=w2t[ci][:, e0:e0 + en],
                            start=(ci == 0), stop=False)
                    nc.tensor.matmul(hp[:, :en],
                                     lhsT=bff[:, bi * 128:(bi + 1) * 128],
                                     rhs=d2t[:, e0:e0 + en],
                                     start=False, stop=False)
                    nc.tensor.matmul(hp[:, :en], lhsT=ones1[:],
                                     rhs=g_row[:, e0:e0 + en],
                                     start=False, stop=True)
                    ysb = tabp.tile([128, 512], F32, tag="ysb")
                    nc.scalar.copy(ysb[:, :en], hp[:, :en])
                    nc.sync.dma_start(y_out[bi * 128:(bi + 1) * 128,
                                            e0:e0 + en], ysb[:, :en])

    nc.compile()
    return nc


def _shard(inputs):
    scores = np.asarray(inputs["scores"], dtype=np.float32)
    adj = np.ascontiguousarray(np.asarray(inputs["exer_conc_adj"],
                                          dtype=np.float32))
    cw = np.ascontiguousarray(np.asarray(inputs["exer_conc_w"],
                                         dtype=np.float32))
    wcc = np.ascontiguousarray(np.asarray(inputs["conc_conc_w"],
                                          dtype=np.float32))
    pote = np.ascontiguousarray(np.asarray(inputs["exer_pote_w"],
                                           dtype=np.float32))
    lambd = np.asarray(inputs["lambd"], dtype=np.float32).reshape(1, E)
    guess = np.asarray(inputs["guess"], dtype=np.float32).reshape(1, E)
    slide = np.asarray(inputs["slide"], dtype=np.float32).reshape(1, E)
    mask = np.asarray(inputs["mask"]).astype(np.float32)
    ids = np.asarray(inputs["exer_ids"]).astype(np.int64)

    # combined gather table: [cw | adj | pote | pad]
    comb = np.zeros((E, ROWW), dtype=GDT_NP)
    comb[:, CW0:CW0 + C] = cw.astype(GDT_NP)
    comb[:, ADJ0:ADJ0 + C] = adj.astype(GDT_NP)
    comb[:, POTE0:POTE0 + P] = pote.astype(GDT_NP)

    in_maps = []
    for k in range(NC_):
        bsl = slice(k * BS, (k + 1) * BS)
        esl = slice(k * ES, (k + 1) * ES)
        # dma_gather wrapped index layout: flat index j (= student*L + l)
        # lives at [j % 16, j // 16], replicated to all 128 partitions.
        flat = ids[bsl].ravel().astype(np.int16)
        iw = np.ascontiguousarray(flat.reshape(-1, 16).T)   # [16, BS*L/16]
        iw = np.tile(iw, (8, 1))                            # [128, BS*L/16]
        ms_t = np.empty((L, 2 * BS), dtype=np.float32)
        ms_t[:, 0::2] = mask[bsl].T
        ms_t[:, 1::2] = scores[bsl].T
        in_maps.append({
            "comb": comb, "idxw": iw,
            "ms_t": np.ascontiguousarray(ms_t).astype(GDT_NP),
            "cw_sh": np.ascontiguousarray(cw[esl]),
            "adj_sh": np.ascontiguousarray(adj[esl]),
            "pote_sh": np.ascontiguousarray(pote[esl]),
            "lam_sh": np.ascontiguousarray(lambd[:, esl]),
            "gue_sh": np.ascontiguousarray(guess[:, esl]),
            "sli_sh": np.ascontiguousarray(slide[:, esl]),
            "wcc": wcc,
        })
    return in_maps


def get_nc():
    if _nc_cache[0] is None:
        _nc_cache[0] = _build()
    return _nc_cache[0]


def run_spmd(inputs, **kw):
    nc = get_nc()
    in_maps = _shard(inputs)
    return bass_utils.run_bass_kernel_spmd(nc, in_maps,
                                           core_ids=list(range(NC_)), **kw)


def assemble(results):
    # A is computed (replicated) on every core in transposed layout.
    A = np.ascontiguousarray(results[0]["A_T_out"].T)
    Y = np.concatenate([results[k]["Y_out"] for k in range(NC_)], axis=1)
    return A, Y


def kernel(**inputs):
    res = run_spmd(inputs)
    return assemble(res.results)
